# revision 2
# baseline (speedup 1.0000x reference)
"""Bahdanau attention kernel for Trainium2 (Bass/Tile), 8-core data-parallel.

Reference computation (per batch row b):
    Wh = W[:, :512]; We = W[:, 512:]
    h_proj = hidden @ Wh.T + b                  [B, 512]
    e_proj = enc @ We.T                         [B, S, 512]
    energy = tanh(h_proj[:, None, :] + e_proj)  [B, S, 512]
    scores = energy @ v                         [B, S]
    scores = where(mask == 0, -1e10, scores)
    out    = softmax(scores, axis=1)            [B, S]

Sharding: data-parallel over batch, 4 rows per core; W/b/v replicated.

v2 dataflow ("natural" orientation — s on partitions, PE does ONLY the
e_proj matmul):
  - enc arrives bf16 via casting SWDGE DMAs in natural layout
    nat[p, ss, e] = enc[s0+ss*128+p, e].
  - the XBAR DMA-transpose engine (InstDmaTransposeAnt, 2-byte dtypes,
    ~14ns/16x128 tile) produces encT[p, j, s] = enc[.., j*128+p] blocks
    [e-part, s-col] -- the PE transposes of v1 are gone entirely.
  - main matmul per s-block of 128: out[s, d] accumulates 8 e-chunk
    matmuls with lhsT = encT block (stationary), rhs = WeT chunk
    [e-part, d-free] (moving, bf16, 512-wide => 1 cyc/row).
  - bias: hpb_bcast[p, bi, d] = h_proj[bi, d] + b[d] replicated across
    partitions (built once via ones-column matmuls); added to PSUM on DVE.
  - tanh on ScalarE (SBUF bf16 -> bf16).
  - v-dot on DVE: scalar_tensor_tensor out=(energy*1.0)*v_bcast with
    accum_out = per-partition free-dim sum = scores column [s-part, 1].
    No PE involvement in scores at all.
  - scores live as scores_all[p, bi, sc, ss]; additive mask is applied in
    the same layout; one tiny PE transpose + a per-bi SBUF->SBUF shuffle
    DMA yields sm[bi, s] rows; softmax epilogue identical to v1
    (reduce_max negate -> ACT Exp bias=-max accum_out=sum -> reciprocal ->
    tensor_scalar_mul).

Per-iteration (bi, sc) steady-state budget: PE 32 matmuls of 512 rows
(~6.8-8.1us), DMA load ~5.8us + XBAR ~3.6us (parallel queues), DVE
~2.6us, ACT ~2.4us => PE-bound at ~55-65us per full pass per core.
"""

import numpy as np

import concourse.bass as bass  # noqa: F401
import concourse.mybir as mybir
import concourse.tile as tile
from concourse import bacc
from concourse.bass_utils import run_bass_kernel_spmd
from concourse.masks import make_identity

F32 = mybir.dt.float32
F32R = mybir.dt.float32r
BF16 = mybir.dt.bfloat16
I32 = mybir.dt.int32
AF = mybir.ActivationFunctionType
ALU = mybir.AluOpType

B, S, E2, DH = 32, 1024, 1024, 512  # batch, seq, 2*enc_hid, dec_hid
NCORES = 8
BL = B // NCORES  # 4 batch rows per core
NEG = -1e10

P = 128
KD = DH // P          # 4 d-chunks
KE = E2 // P          # 8 e-chunks
KIN = (DH + E2) // P  # 12 input-dim chunks of W
SC = 512              # s-chunk per iteration
NSC = S // SC         # 2 s-chunks per batch row
SSB = SC // P         # 4 s-subblocks of 128 per s-chunk


def _build_kernel(reps=1):
    nc = bacc.Bacc(
        "TRN2",
        target_bir_lowering=False,
        debug=False,
        enable_asserts=False,
        num_devices=NCORES,
    )
    hid_d = nc.dram_tensor("hidden", [BL, DH], F32, kind="ExternalInput").ap()
    enc_d = nc.dram_tensor("enc", [BL, S, E2], F32, kind="ExternalInput").ap()
    mask_d = nc.dram_tensor("mask", [BL, S], I32, kind="ExternalInput").ap()
    w_d = nc.dram_tensor("W", [DH, DH + E2], F32, kind="ExternalInput").ap()
    b_d = nc.dram_tensor("b", [DH], F32, kind="ExternalInput").ap()
    v_d = nc.dram_tensor("v", [DH], F32, kind="ExternalInput").ap()
    out_d = nc.dram_tensor("out", [BL, S], F32, kind="ExternalOutput").ap()

    with tile.TileContext(nc) as tc:
        with (
            tc.tile_pool(name="const", bufs=1) as constp,
            tc.tile_pool(name="nat", bufs=3) as natp,
            tc.tile_pool(name="encT", bufs=2) as encp,
            tc.tile_pool(name="tmp", bufs=3) as tmpp,
            tc.tile_pool(name="energy", bufs=3) as enp,
            tc.tile_pool(name="ven", bufs=2) as venp,
            tc.tile_pool(name="scores", bufs=2) as scop,
            tc.tile_pool(name="small", bufs=2) as smp,
            tc.tile_pool(name="pt", bufs=2, space="PSUM") as ptp,
            tc.tile_pool(name="pmm", bufs=4, space="PSUM") as mmp,
            tc.tile_pool(name="psetup", bufs=1, space="PSUM") as setp,
            tc.tile_pool(name="pepi", bufs=1, space="PSUM") as epip,
        ):
            # ---------------- setup ----------------
            ident = constp.tile([P, P], F32)
            make_identity(nc, ident[:])
            ident_r_t = constp.tile([P, P], F32R)
            nc.vector.tensor_copy(ident_r_t[:], ident[:])
            ident_r = ident_r_t[:]

            # f32 loads (f32r is a bitcast); casts to bf16 happen on-chip
            # during the PSUM->SBUF copies below.
            w_nat = constp.tile([P, KD, DH + E2], F32R)  # [p, j, c]: W[j*128+p, c]
            nc.gpsimd.dma_start(w_nat[:], w_d.rearrange("(j p) c -> p j c", p=P))
            hid_sb = constp.tile([BL, DH], F32R)
            nc.gpsimd.dma_start(hid_sb[:], hid_d)
            b_row_f = constp.tile([1, DH], F32R)
            nc.gpsimd.dma_start(b_row_f[:], b_d.rearrange("(o d) -> o d", o=1))
            v_row_f = constp.tile([1, DH], F32R)
            nc.gpsimd.dma_start(v_row_f[:], v_d.rearrange("(o d) -> o d", o=1))
            # mask in the scores layout [p, bi, sc, ss], s = sc*512+ss*128+p
            mask_nat = constp.tile([P, BL, NSC, SSB], I32)
            nc.sync.dma_start(
                mask_nat[:],
                mask_d.rearrange("bi (sc ss p) -> p bi sc ss", p=P, sc=NSC),
            )

            ones_f = constp.tile([1, P], F32)
            nc.vector.memset(ones_f[:], 1.0)
            ones_r = constp.tile([1, P], F32R)
            nc.vector.tensor_copy(ones_r[:], ones_f[:])

            # additive mask in scores layout: 0 where mask==1, -1e10 where 0
            maskneg_nat = constp.tile([P, BL, NSC, SSB], F32)
            nc.scalar.activation(
                maskneg_nat[:], mask_nat[:], AF.Copy, bias=-1e10, scale=1e10
            )

            # WT[p, i, c] = W[c, i*128+p]  (full transpose of W, cast to bf16)
            wT = constp.tile([P, KIN, DH], BF16)
            for i in range(KIN):
                pt = ptp.tile([P, SC], F32R, tag="pt")
                for j in range(KD):
                    nc.tensor.transpose(
                        pt[:, j * P : (j + 1) * P],
                        w_nat[:, j, i * P : (i + 1) * P],
                        ident_r,
                    )
                if i % 2:
                    nc.vector.tensor_copy(wT[:, i, :], pt[:].bitcast(F32))
                else:
                    nc.scalar.copy(wT[:, i, :], pt[:].bitcast(F32))

            # hidden transposed: hidT[p, kk, m] = hidden[m, kk*128+p], bf16
            hidT = constp.tile([P, KD, BL], BF16)
            pt = ptp.tile([P, SC], F32R, tag="pt")
            for kk in range(KD):
                nc.tensor.transpose(
                    pt[:, kk * BL : (kk + 1) * BL],
                    hid_sb[:, kk * P : (kk + 1) * P],
                    ident_r[0:BL, 0:BL],
                )
            nc.scalar.copy(hidT[:], pt[:, : KD * BL].bitcast(F32))

            # h_proj + b, one batch row at a time into partition 0, then
            # broadcast to all 128 partitions with a ones-column matmul
            # (keeps every matmul operand at base partition 0).
            b_row_bf = constp.tile([1, DH], BF16)
            nc.vector.tensor_copy(b_row_bf[:], b_row_f[:].bitcast(F32))
            ones_bf = constp.tile([1, P], BF16)
            nc.vector.tensor_copy(ones_bf[:], ones_f[:])
            hpb_bcast = constp.tile([P, BL, DH], F32)
            for bi in range(BL):
                ph_ps = setp.tile([P, DH], F32, tag="set")
                for kk in range(KD):
                    nc.tensor.matmul(
                        ph_ps[0:1, :],
                        hidT[:, kk, bi : bi + 1],
                        wT[:, kk, :],
                        start=(kk == 0),
                        stop=False,
                    )
                nc.tensor.matmul(
                    ph_ps[0:1, :], ones_bf[:, 0:1], b_row_bf[:],
                    start=False, stop=True,
                )
                ph_row = constp.tile([1, DH], F32R, tag=f"phrow{bi}")
                nc.vector.tensor_copy(ph_row[:], ph_ps[0:1, :])
                hb_ps = setp.tile([P, DH], F32, tag="set")
                nc.tensor.matmul(
                    hb_ps[:], ones_r[:], ph_row[:], start=True, stop=True
                )
                if bi % 2:
                    nc.vector.tensor_copy(hpb_bcast[:, bi, :], hb_ps[:])
                else:
                    nc.scalar.copy(hpb_bcast[:, bi, :], hb_ps[:])

            # v_bcast[p, d] = v[d] for every partition p (bf16)
            v_bcast = constp.tile([P, DH], BF16)
            vb_ps = setp.tile([P, DH], F32, tag="set")
            nc.tensor.matmul(
                vb_ps[:], ones_r[:], v_row_f[:], start=True, stop=True
            )
            nc.vector.tensor_copy(v_bcast[:], vb_ps[:])

            # ---------------- main loop ----------------
            # reps>1 repeats the identical full pass (main loop + softmax
            # epilogue) for slope-based HW timing; output unchanged.
            for _rep in range(reps):
                scores_all = scop.tile([P, BL, NSC, SSB], F32, tag="sc")
                for bi in range(BL):
                    for sc in range(NSC):
                        # enc chunk, bf16, natural layout (casting SWDGE DMA)
                        nat = natp.tile([P, SSB, E2], BF16, tag="nat")
                        nc.gpsimd.dma_start(
                            nat[:],
                            enc_d[bi, sc * SC : (sc + 1) * SC, :].rearrange(
                                "(ss p) e -> p ss e", p=P
                            ),
                        )
                        # XBAR DMA transpose: encT[p, ss*8+ec, s] =
                        #   enc[s0+ss*128+s, ec*128+p]
                        encT = encp.tile([P, SSB * KE, P], BF16, tag="encT")
                        nc.sync.dma_start(encT[:], nat[:], transpose=True)

                        for ss in range(SSB):
                            pm = mmp.tile([P, DH], F32, tag="pm")
                            for ec in range(KE):
                                nc.tensor.matmul(
                                    pm[:],
                                    encT[:, ss * KE + ec, :],
                                    wT[:, KD + ec, :],
                                    start=(ec == 0),
                                    stop=(ec == KE - 1),
                                )
                            # + h_proj + b (free-dim bias) on DVE, cast bf16
                            tmp = tmpp.tile([P, DH], BF16, tag="tmp")
                            nc.vector.tensor_add(
                                tmp[:], pm[:], hpb_bcast[:, bi, :]
                            )
                            energy = enp.tile([P, DH], BF16, tag="en")
                            nc.scalar.activation(energy[:], tmp[:], AF.Tanh)
                            # v-dot on DVE: accum_out = sum_d energy*v
                            ven = venp.tile([P, DH], BF16, tag="ven")
                            nc.vector.scalar_tensor_tensor(
                                ven[:],
                                energy[:],
                                1.0,
                                v_bcast[:],
                                ALU.mult,
                                ALU.mult,
                                accum_out=scores_all[:, bi, sc, ss : ss + 1],
                            )

                # ---------------- masked softmax epilogue ----------------
                scores_m = scop.tile([P, BL * NSC * SSB], F32, tag="scm")
                nc.vector.tensor_add(
                    scores_m[:],
                    scores_all[:].rearrange("p a b c -> p (a b c)"),
                    maskneg_nat[:].rearrange("p a b c -> p (a b c)"),
                )
                # transpose to [(bi,sc,ss), s-mod-128]
                sc_t = epip.tile([BL * NSC * SSB, P], F32, tag="sct")
                nc.tensor.transpose(sc_t[:], scores_m[:], ident[:])
                sm32 = smp.tile([BL * NSC * SSB, P], F32, tag="sm32")
                nc.vector.tensor_copy(sm32[:], sc_t[:])
                # gather each batch row's 8 blocks into a single row
                sm = smp.tile([BL, S], F32, tag="sm")
                for bi in range(BL):
                    nc.scalar.dma_start(
                        sm[bi : bi + 1, :],
                        sm32[bi * NSC * SSB : (bi + 1) * NSC * SSB, :],
                    )
                negmax = smp.tile([BL, 1], F32, tag="negmax")
                nc.vector.tensor_reduce(
                    negmax[:], sm[:], axis=mybir.AxisListType.X,
                    op=mybir.AluOpType.max, negate=True,
                )
                expv = smp.tile([BL, S], F32, tag="expv")
                sumexp = smp.tile([BL, 1], F32, tag="sumexp")
                nc.scalar.activation(
                    expv[:], sm[:], AF.Exp, bias=negmax[:], accum_out=sumexp[:]
                )
                rec = smp.tile([BL, 1], F32, tag="rec")
                nc.vector.reciprocal(rec[:], sumexp[:])
                outsb = smp.tile([BL, S], F32, tag="outsb")
                nc.vector.tensor_scalar_mul(outsb[:], expv[:], rec[:])
                nc.scalar.dma_start(out_d, outsb[:])

    nc.compile()
    return nc


_NC_CACHE = None
LAST_RESULTS = None


def kernel(hidden, encoder_outputs, mask, W, b, v, _trace=False):
    global _NC_CACHE, LAST_RESULTS
    if _NC_CACHE is None:
        _NC_CACHE = _build_kernel()
    nc = _NC_CACHE

    hidden = np.ascontiguousarray(np.asarray(hidden, dtype=np.float32))
    enc = np.ascontiguousarray(np.asarray(encoder_outputs, dtype=np.float32))
    mask = np.ascontiguousarray(np.asarray(mask, dtype=np.int32))
    W = np.ascontiguousarray(np.asarray(W, dtype=np.float32))
    b = np.ascontiguousarray(np.asarray(b, dtype=np.float32))
    v = np.ascontiguousarray(np.asarray(v, dtype=np.float32))

    in_maps = []
    for c in range(NCORES):
        sl = slice(c * BL, (c + 1) * BL)
        in_maps.append(
            {
                "hidden": np.ascontiguousarray(hidden[sl]),
                "enc": np.ascontiguousarray(enc[sl]),
                "mask": np.ascontiguousarray(mask[sl]),
                "W": W,
                "b": b,
                "v": v,
            }
        )

    res = run_bass_kernel_spmd(
        nc, in_maps, core_ids=list(range(NCORES)), trace=_trace
    )
    LAST_RESULTS = res
    return np.concatenate([r["out"] for r in res.results], axis=0)


def bench(in_maps=None, iters=30, inputs=None, reps=1, nc=None):
    """Time repeated executions with device-resident inputs (amortizes the
    axon transfer/dispatch overhead). Returns (sec/iter, core0 output).

    iters > 0: async pipelined loop (block once at the end).
    iters < 0: -iters fully-blocking trials, return the min.
    """
    import time

    import jax
    import numpy as np_
    from jax.experimental.shard_map import shard_map
    from jax.sharding import Mesh, NamedSharding, PartitionSpec

    import concourse.mybir as mybir
    from concourse.bass2jax import (
        _bass_exec_p,
        install_neuronx_cc_hook,
        partition_id_tensor,
    )

    global _NC_CACHE
    if nc is None:
        if reps == 1:
            if _NC_CACHE is None:
                _NC_CACHE = _build_kernel()
            nc = _NC_CACHE
        else:
            nc = _build_kernel(reps)
    install_neuronx_cc_hook()

    if in_maps is None:
        assert inputs is not None
        hidden = np_.asarray(inputs["hidden"], dtype=np_.float32)
        enc = np_.asarray(inputs["encoder_outputs"], dtype=np_.float32)
        mask = np_.asarray(inputs["mask"], dtype=np_.int32)
        W = np_.asarray(inputs["W"], dtype=np_.float32)
        b = np_.asarray(inputs["b"], dtype=np_.float32)
        v = np_.asarray(inputs["v"], dtype=np_.float32)
        in_maps = []
        for c in range(NCORES):
            sl = slice(c * BL, (c + 1) * BL)
            in_maps.append({"hidden": hidden[sl], "enc": enc[sl], "mask": mask[sl],
                            "W": W, "b": b, "v": v})

    partition_name = nc.partition_id_tensor.name if nc.partition_id_tensor else None
    in_names, out_names, out_avals, zero_outs = [], [], [], []
    for alloc in nc.m.functions[0].allocations:
        if not isinstance(alloc, mybir.MemoryLocationSet):
            continue
        name = alloc.memorylocations[0].name
        if alloc.kind == "ExternalInput":
            if name != partition_name:
                in_names.append(name)
        elif alloc.kind == "ExternalOutput":
            shape = tuple(alloc.tensor_shape)
            dtype = mybir.dt.np(alloc.dtype)
            out_names.append(name)
            out_avals.append(jax.core.ShapedArray(shape, dtype))
            zero_outs.append(np_.zeros(shape, dtype))
    n_params = len(in_names)
    n_outs = len(out_avals)
    in_names.extend(out_names)
    if partition_name is not None:
        in_names.append(partition_name)

    def _body(*args):
        operands = list(args)
        if partition_name is not None:
            operands.append(partition_id_tensor())
        outs = _bass_exec_p.bind(
            *operands,
            out_avals=tuple(out_avals),
            in_names=tuple(in_names),
            out_names=tuple(out_names),
            lowering_input_output_aliases=(),
            sim_require_finite=True,
            sim_require_nnan=True,
            nc=nc,
        )
        return tuple(outs)

    devices = jax.devices()[:NCORES]
    mesh = Mesh(np_.asarray(devices), ("core",))
    in_specs = (PartitionSpec("core"),) * (n_params + n_outs)
    out_specs = (PartitionSpec("core"),) * n_outs
    # no donation so device inputs survive across iterations
    sharded = jax.jit(
        shard_map(_body, mesh=mesh, in_specs=in_specs, out_specs=out_specs,
                  check_rep=False),
        keep_unused=True,
    )
    shard = NamedSharding(mesh, PartitionSpec("core"))
    concat_in = [
        jax.device_put(
            np_.concatenate([np_.asarray(in_maps[c][nm]) for c in range(NCORES)],
                            axis=0),
            shard,
        )
        for nm in in_names[:n_params]
    ]
    concat_zeros = [
        jax.device_put(np_.zeros((NCORES * z.shape[0], *z.shape[1:]), z.dtype), shard)
        for z in zero_outs
    ]
    # warmup + correctness reference output
    outs = sharded(*concat_in, *concat_zeros)
    jax.block_until_ready(outs)
    if iters < 0:
        best = None
        for _ in range(-iters):
            t0 = time.time()
            outs = sharded(*concat_in, *concat_zeros)
            jax.block_until_ready(outs)
            dt = time.time() - t0
            best = dt if best is None else min(best, dt)
        return best, np_.asarray(outs[0])
    t0 = time.time()
    for _ in range(iters):
        outs = sharded(*concat_in, *concat_zeros)
    jax.block_until_ready(outs)
    dt = (time.time() - t0) / iters
    out_np = np_.asarray(outs[0])
    return dt, out_np
